# revision 3
# baseline (speedup 1.0000x reference)
"""Trainium2 Bass kernel for nn_NeuralALU (batched byte-encoded 32-bit add).

The reference network computes, per batch element, a chain of table-lookup
matmuls + sharp softmaxes (scale=100) over exactly-one-hot byte encodings.
Because the inputs are exact one-hots, the float pipeline collapses to a
discrete algorithm (validated to ~1e-22 rel-err):

  s = a + b                      per 256-wide byte block (<=2 nonzeros)
  z = dot(s, code2)              code2 packs lo/hi nibble sums of 2 bytes
                                 into four 6-bit fields (exact in f32)
  xnib[n] in [0,30]              per-nibble sums, carry order lo0,hi0,...
  soft carry chain:  Y_n = x_n + c_n,  c_{n+1} = clamp(Y_n - 15, 0, 1)
                     (kept shifted: c~ = c + 15 so both steps are 1 op)
  soft dist:  ymod = Y - 16*[Y >= 15.75]
              s17[j] = relu(1 - |ymod - j|),  j = 0..16   (17-wide)
              dist[k] = s17[k],  dist[0] += s17[16]       (wraparound)
       This one triangle kernel reproduces onehot(U)*(1-P/2) +
       onehot((U+1)%16)*(P/2) for carry states c in {0, 0.5, 1}.
  out byte row [256] = outer(h_dist, l_dist) flattened

Engine plan (per core, pure data-parallel over batch):
  - s = a+b is computed BY THE DMA ENGINE: a-batch loads on HWDGE (SP
    ring), b-batch loads on SWDGE with accum_op=add (CCE inline add).
  - DVE: packed dots, field extraction, carry chain, soft dists, a
    minority of the outer products.
  - Pool (gpsimd): majority of outer products.
  - ACT (scalar): issues output stores on its own HWDGE ring so input
    loads and output stores never share a FIFO.
"""

import numpy as np

import concourse.bass as bass
import concourse.bacc as bacc
import concourse.mybir as mybir
from concourse.tile import TileContext
from concourse.bass_utils import run_bass_kernel_spmd

N_CORES = 8
B_FULL = 32768
ROWS = B_FULL // N_CORES  # 4096 rows per core
F = 1024  # 4 bytes x 256 one-hot
P = 128
NTC = 16       # tiles per chunk
BATCH = 4      # tiles per DMA batch
POOL_OUTER_PER_BATCH = 3  # of BATCH outer products, how many run on Pool

FP = mybir.dt.float32
I32 = mybir.dt.int32


def _const_tables():
    k = np.arange(256)
    nib = ((k % 16) + 64.0 * (k // 16)).astype(np.float32)  # <= 975
    # two bytes per dot: second byte's fields scaled by 4096 (sums stay
    # exact in f32: max 2*975*4096 + 2*975 < 2^23)
    code2 = np.concatenate([nib, nib * 4096.0])  # [512]
    code2 = np.broadcast_to(code2, (P, 512)).copy()
    iota17 = np.broadcast_to(np.arange(17, dtype=np.float32), (P, 17)).copy()
    return code2, iota17


def build_nc(rows=ROWS):
    nt = rows // P
    assert nt % NTC == 0 and NTC % BATCH == 0
    n_chunks = nt // NTC
    nbat = NTC // BATCH

    AL = mybir.AluOpType
    nc = bacc.Bacc()
    a_d = nc.declare_dram_parameter("a", [rows, F], FP, isOutput=False)
    b_d = nc.declare_dram_parameter("b", [rows, F], FP, isOutput=False)
    code_d = nc.declare_dram_parameter("code2", [P, 512], FP, isOutput=False)
    iota_d = nc.declare_dram_parameter("iota17", [P, 17], FP, isOutput=False)
    out_d = nc.declare_dram_parameter("out", [rows, F], FP, isOutput=True)

    a_v = a_d[:, :].rearrange("(bt t p) f -> bt p t f", t=BATCH, p=P)
    b_v = b_d[:, :].rearrange("(bt t p) f -> bt p t f", t=BATCH, p=P)
    out_v = out_d[:, :].rearrange("(bt t p) f -> bt p t f", t=BATCH, p=P)

    with TileContext(nc) as tc:
        with (
            tc.tile_pool(name="consts", bufs=1) as cpool,
            tc.tile_pool(name="io", bufs=4) as iopool,
            tc.tile_pool(name="scratch", bufs=4) as scpool,
            tc.tile_pool(name="arrs", bufs=2) as apool,
            tc.tile_pool(name="outp", bufs=3) as opool,
        ):
            code_raw = cpool.tile([P, 512], FP, tag="code_raw")
            code2 = cpool.tile([P, 512], FP, tag="code2")
            iota_raw = cpool.tile([P, 17], FP, tag="iota_raw")
            iota17 = cpool.tile([P, 17], FP, tag="iota17")
            nc.sync.dma_start(code_raw[:, :], code_d[:, :])
            nc.sync.dma_start(iota_raw[:, :], iota_d[:, :])
            # pre-touch consts on DVE so compute ops only wait on DVE state
            nc.vector.tensor_copy(code2[:, :], code_raw[:, :])
            nc.vector.tensor_copy(iota17[:, :], iota_raw[:, :])

            for ch in range(n_chunks):
                z2 = apool.tile([P, 2 * NTC], FP, tag="z2")
                z2_i = apool.tile([P, 2 * NTC], I32, tag="z2i")
                xnib_i = apool.tile([P, 8 * NTC], I32, tag="xnibi")
                xnib = apool.tile([P, 8 * NTC], FP, tag="xnib")
                chist = apool.tile([P, 9 * NTC], FP, tag="chist")
                y_all = apool.tile([P, 8 * NTC], FP, tag="yall")
                wrap = apool.tile([P, 8 * NTC], FP, tag="wrap")
                ymod = apool.tile([P, 8 * NTC], FP, tag="ymod")
                dtile = apool.tile([P, 8 * NTC * 17], FP, tag="dtile")
                s17 = apool.tile([P, 8 * NTC * 17], FP, tag="s17")

                # ---- phase 1: batched loads, s=a+b in the DMA, dots ----
                s_bats = []
                for bt in range(nbat):
                    g = ch * nbat + bt
                    s_b = iopool.tile([P, BATCH * F], FP, tag="sb")
                    s_bv = s_b[:, :].rearrange("p (t f) -> p t f", t=BATCH)
                    nc.sync.dma_start(s_bv, a_v[g])
                    nc.gpsimd.dma_start(s_bv, b_v[g], accum_op=AL.add)
                    s_bats.append(s_b)
                    for ti in range(BATCH):
                        t = bt * BATCH + ti
                        for i2 in range(2):
                            prod = scpool.tile([P, 512], FP, tag="prod")
                            nc.vector.scalar_tensor_tensor(
                                out=prod[:, :],
                                in0=s_b[:, ti * F + i2 * 512 : ti * F + (i2 + 1) * 512],
                                scalar=1.0,
                                in1=code2[:, :],
                                op0=AL.mult,
                                op1=AL.mult,
                                accum_out=z2[:, i2 * NTC + t : i2 * NTC + t + 1],
                            )

                # ---- phase 2: unpack four 6-bit fields -> xnib ----
                nc.vector.tensor_copy(z2_i[:, :], z2[:, :])  # f32 -> i32 exact
                z2_v = z2_i[:, :].rearrange("p (i2 t) -> p i2 t", t=NTC)
                xn_v = xnib_i[:, :].rearrange("p (i2 k t) -> p i2 k t", i2=2, k=4)
                nc.vector.tensor_scalar(
                    out=xn_v[:, :, 0, :], in0=z2_v, scalar1=63, scalar2=None,
                    op0=AL.bitwise_and,
                )
                nc.vector.tensor_scalar(
                    out=xn_v[:, :, 1, :], in0=z2_v, scalar1=6, scalar2=63,
                    op0=AL.logical_shift_right, op1=AL.bitwise_and,
                )
                nc.vector.tensor_scalar(
                    out=xn_v[:, :, 2, :], in0=z2_v, scalar1=12, scalar2=63,
                    op0=AL.logical_shift_right, op1=AL.bitwise_and,
                )
                nc.vector.tensor_scalar(
                    out=xn_v[:, :, 3, :], in0=z2_v, scalar1=18, scalar2=None,
                    op0=AL.logical_shift_right,
                )
                nc.vector.tensor_copy(xnib[:, :], xnib_i[:, :])  # i32 -> f32

                # ---- phase 3: carry chain, shifted carry c~ = c + 15 ----
                nc.vector.memset(chist[:, 0:NTC], 15.5)
                for n in range(8):
                    y_n = y_all[:, n * NTC : (n + 1) * NTC]
                    nc.vector.scalar_tensor_tensor(
                        out=y_n, in0=xnib[:, n * NTC : (n + 1) * NTC],
                        scalar=-15.0, in1=chist[:, n * NTC : (n + 1) * NTC],
                        op0=AL.add, op1=AL.add,
                    )
                    nc.vector.tensor_scalar(
                        out=chist[:, (n + 1) * NTC : (n + 2) * NTC], in0=y_n,
                        scalar1=15.0, scalar2=16.0, op0=AL.max, op1=AL.min,
                    )

                # ---- phase 4: wrap to ymod in [0, 15.5] ----
                nc.vector.tensor_scalar(
                    out=wrap[:, :], in0=y_all[:, :], scalar1=15.75, scalar2=None,
                    op0=AL.is_ge,
                )
                nc.vector.scalar_tensor_tensor(
                    out=ymod[:, :], in0=wrap[:, :], scalar=-16.0, in1=y_all[:, :],
                    op0=AL.mult, op1=AL.add,
                )

                # ---- phase 5: soft dists s17 = relu(1 - |ymod - j|) ----
                # |.| and relu run on the otherwise-idle ACT engine
                G = 8 * NTC
                d_v = dtile[:, :].rearrange("p (g k) -> p g k", k=17)
                s_v = s17[:, :].rearrange("p (g k) -> p g k", k=17)
                ymod_b = ymod[:, :, None].broadcast_to([P, G, 17])
                iota_b = iota17[:, None, :].broadcast_to([P, G, 17])
                nc.vector.tensor_tensor(d_v, ymod_b, iota_b, op=AL.subtract)
                nc.scalar.activation(
                    dtile[:, :], dtile[:, :], mybir.ActivationFunctionType.Abs,
                )
                nc.scalar.activation(
                    s17[:, :], dtile[:, :], mybir.ActivationFunctionType.Relu,
                    bias=1.0, scale=-1.0,
                )
                # wraparound fold: dist[0] += s17[16]
                nc.vector.tensor_add(
                    s_v[:, :, 0:1], s_v[:, :, 0:1], s_v[:, :, 16:17]
                )

                # ---- phase 6: outer products + stores ----
                s5 = s17[:, :].rearrange(
                    "p (i two t k) -> p i two t k", two=2, t=NTC, k=17
                )
                for bt in range(nbat):
                    g = ch * nbat + bt
                    o4 = opool.tile([P, BATCH * F], FP, tag="o4")
                    for ti in range(BATCH):
                        t = bt * BATCH + ti
                        o_vv = o4[:, ti * F : (ti + 1) * F].rearrange(
                            "p (i h l) -> p i h l", h=16, l=16
                        )
                        h_b = s5[:, :, 1, t, 0:16][:, :, :, None].broadcast_to(
                            [P, 4, 16, 16])
                        l_b = s5[:, :, 0, t, 0:16][:, :, None, :].broadcast_to(
                            [P, 4, 16, 16])
                        eng = nc.gpsimd if ti < POOL_OUTER_PER_BATCH else nc.vector
                        eng.tensor_mul(o_vv, h_b, l_b)
                    o4_v = o4[:, :].rearrange("p (t f) -> p t f", t=BATCH)
                    nc.scalar.dma_start(out_v[g], o4_v)

    nc.finalize()
    return nc


_NC_CACHE = {}
LAST_RESULT = None


def kernel(**inputs) -> np.ndarray:
    global LAST_RESULT
    a = np.ascontiguousarray(np.asarray(inputs["a"], dtype=np.float32)).reshape(B_FULL, F)
    b = np.ascontiguousarray(np.asarray(inputs["b"], dtype=np.float32)).reshape(B_FULL, F)
    code2, iota17 = _const_tables()

    if ROWS not in _NC_CACHE:
        _NC_CACHE[ROWS] = build_nc(ROWS)
    nc = _NC_CACHE[ROWS]

    in_maps = []
    for c in range(N_CORES):
        in_maps.append({
            "a": a[c * ROWS : (c + 1) * ROWS],
            "b": b[c * ROWS : (c + 1) * ROWS],
            "code2": code2,
            "iota17": iota17,
        })
    res = run_bass_kernel_spmd(nc, in_maps, core_ids=list(range(N_CORES)))
    LAST_RESULT = res
    out = np.concatenate([r["out"] for r in res.results], axis=0)
    return out.reshape(B_FULL, 4, 256)


# revision 5
# speedup vs baseline: 1.1645x; 1.1645x over previous
"""Trainium2 Bass kernel for nn_NeuralALU (batched byte-encoded 32-bit add).

The reference network computes, per batch element, a chain of table-lookup
matmuls + sharp softmaxes (scale=100) over exactly-one-hot byte encodings.
Because the inputs are exact one-hots, the float pipeline collapses to a
discrete algorithm (validated to ~1e-22 rel-err):

  s = a + b                      per 256-wide byte block (<=2 nonzeros)
  z = dot(s, code2)              code2 packs lo/hi nibble sums of 2 bytes
                                 into four 6-bit fields (exact in f32)
  xnib[n] in [0,30]              per-nibble sums, carry order lo0,hi0,...
  soft carry chain:  Y_n = x_n + c_n,  c_{n+1} = clamp(Y_n - 15, 0, 1)
                     (kept shifted: c~ = c + 15 so both steps are 1 op)
  soft dist:  ymod = Y - 16*[Y >= 15.75]
              s17[j] = relu(1 - |ymod - j|),  j = 0..16   (17-wide)
              dist[k] = s17[k],  dist[0] += s17[16]       (wraparound)
       This one triangle kernel reproduces onehot(U)*(1-P/2) +
       onehot((U+1)%16)*(P/2) for carry states c in {0, 0.5, 1}.
  out byte row [256] = outer(h_dist, l_dist) flattened

Engine plan (per core, pure data-parallel over batch):
  - s = a+b is computed BY THE DMA ENGINE: a-batch loads on HWDGE (SP
    ring), b-batch loads on SWDGE with accum_op=add (CCE inline add).
  - DVE: packed dots, field extraction, carry chain, soft dists, a
    minority of the outer products.
  - Pool (gpsimd): majority of outer products.
  - ACT (scalar): issues output stores on its own HWDGE ring so input
    loads and output stores never share a FIFO.
"""

import numpy as np

import concourse.bass as bass
import concourse.bacc as bacc
import concourse.mybir as mybir
from concourse.tile import TileContext
from concourse.bass_utils import run_bass_kernel_spmd

N_CORES = 8
B_FULL = 32768
ROWS = B_FULL // N_CORES  # 4096 rows per core
F = 1024  # 4 bytes x 256 one-hot
P = 128
NTC = 16       # tiles per chunk
BATCH = 4      # tiles per DMA batch
POOL_OUTER_PER_BATCH = 4  # of BATCH outer products, how many run on Pool

FP = mybir.dt.float32
I32 = mybir.dt.int32


def _const_tables():
    k = np.arange(256)
    nib = ((k % 16) + 64.0 * (k // 16)).astype(np.float32)  # <= 975
    # two bytes per dot: second byte's fields scaled by 4096 (sums stay
    # exact in f32: max 2*975*4096 + 2*975 < 2^23)
    code2 = np.concatenate([nib, nib * 4096.0])  # [512]
    code2 = np.broadcast_to(code2, (P, 512)).copy()
    iota17 = np.broadcast_to(np.arange(17, dtype=np.float32), (P, 17)).copy()
    return code2, iota17


def build_nc(rows=ROWS):
    nt = rows // P
    assert nt % NTC == 0 and NTC % BATCH == 0
    n_chunks = nt // NTC
    nbat = NTC // BATCH

    AL = mybir.AluOpType
    nc = bacc.Bacc()
    a_d = nc.declare_dram_parameter("a", [rows, F], FP, isOutput=False)
    b_d = nc.declare_dram_parameter("b", [rows, F], FP, isOutput=False)
    code_d = nc.declare_dram_parameter("code2", [P, 512], FP, isOutput=False)
    iota_d = nc.declare_dram_parameter("iota17", [P, 17], FP, isOutput=False)
    out_d = nc.declare_dram_parameter("out", [rows, F], FP, isOutput=True)

    a_v = a_d[:, :].rearrange("(bt t p) f -> bt p t f", t=BATCH, p=P)
    b_v = b_d[:, :].rearrange("(bt t p) f -> bt p t f", t=BATCH, p=P)
    out_v = out_d[:, :].rearrange("(bt t p) f -> bt p t f", t=BATCH, p=P)

    with TileContext(nc) as tc:
        with (
            tc.tile_pool(name="consts", bufs=1) as cpool,
            tc.tile_pool(name="io", bufs=3) as iopool,
            tc.tile_pool(name="scratch", bufs=4) as scpool,
            tc.tile_pool(name="arrs", bufs=2) as apool,
            tc.tile_pool(name="outp", bufs=2) as opool,
        ):
            code_raw = cpool.tile([P, 512], FP, tag="code_raw")
            code2 = cpool.tile([P, 512], FP, tag="code2")
            iota_raw = cpool.tile([P, 17], FP, tag="iota_raw")
            iota17 = cpool.tile([P, 17], FP, tag="iota17")
            nc.sync.dma_start(code_raw[:, :], code_d[:, :])
            nc.sync.dma_start(iota_raw[:, :], iota_d[:, :])
            # pre-touch consts on DVE so compute ops only wait on DVE state
            nc.vector.tensor_copy(code2[:, :], code_raw[:, :])
            nc.vector.tensor_copy(iota17[:, :], iota_raw[:, :])

            for ch in range(n_chunks):
                z2 = apool.tile([P, 2 * NTC], FP, tag="z2")
                z2_i = apool.tile([P, 2 * NTC], I32, tag="z2i")
                xnib_i = apool.tile([P, 8 * NTC], I32, tag="xnibi")
                xnib = apool.tile([P, 8 * NTC], FP, tag="xnib")
                chist = apool.tile([P, 9 * NTC], FP, tag="chist")
                y_all = apool.tile([P, 8 * NTC], FP, tag="yall")
                wrap = apool.tile([P, 8 * NTC], FP, tag="wrap")
                ymod = apool.tile([P, 8 * NTC], FP, tag="ymod")
                dtile = apool.tile([P, 8 * NTC * 17], FP, tag="dtile")
                s17 = apool.tile([P, 8 * NTC * 17], FP, tag="s17")

                # ---- phase 1: batched loads, s = a+b (DVE, in-place), dots ----
                for bt in range(nbat):
                    g = ch * nbat + bt
                    a_b = iopool.tile([P, BATCH * F], FP, tag="ab")
                    b_b = iopool.tile([P, BATCH * F], FP, tag="bb")
                    nc.sync.dma_start(
                        a_b[:, :].rearrange("p (t f) -> p t f", t=BATCH), a_v[g])
                    nc.sync.dma_start(
                        b_b[:, :].rearrange("p (t f) -> p t f", t=BATCH), b_v[g])
                    for ti in range(BATCH):
                        t = bt * BATCH + ti
                        sl = slice(ti * F, (ti + 1) * F)
                        nc.vector.tensor_add(a_b[:, sl], a_b[:, sl], b_b[:, sl])
                        for i2 in range(2):
                            prod = scpool.tile([P, 512], FP, tag="prod")
                            nc.vector.scalar_tensor_tensor(
                                out=prod[:, :],
                                in0=a_b[:, ti * F + i2 * 512 : ti * F + (i2 + 1) * 512],
                                scalar=1.0,
                                in1=code2[:, :],
                                op0=AL.mult,
                                op1=AL.mult,
                                accum_out=z2[:, i2 * NTC + t : i2 * NTC + t + 1],
                            )

                # ---- phase 2: unpack four 6-bit fields -> xnib ----
                nc.vector.tensor_copy(z2_i[:, :], z2[:, :])  # f32 -> i32 exact
                z2_v = z2_i[:, :].rearrange("p (i2 t) -> p i2 t", t=NTC)
                xn_v = xnib_i[:, :].rearrange("p (i2 k t) -> p i2 k t", i2=2, k=4)
                nc.vector.tensor_scalar(
                    out=xn_v[:, :, 0, :], in0=z2_v, scalar1=63, scalar2=None,
                    op0=AL.bitwise_and,
                )
                nc.vector.tensor_scalar(
                    out=xn_v[:, :, 1, :], in0=z2_v, scalar1=6, scalar2=63,
                    op0=AL.logical_shift_right, op1=AL.bitwise_and,
                )
                nc.vector.tensor_scalar(
                    out=xn_v[:, :, 2, :], in0=z2_v, scalar1=12, scalar2=63,
                    op0=AL.logical_shift_right, op1=AL.bitwise_and,
                )
                nc.vector.tensor_scalar(
                    out=xn_v[:, :, 3, :], in0=z2_v, scalar1=18, scalar2=None,
                    op0=AL.logical_shift_right,
                )
                nc.vector.tensor_copy(xnib[:, :], xnib_i[:, :])  # i32 -> f32

                # ---- phase 3: carry chain, shifted carry c~ = c + 15 ----
                nc.vector.memset(chist[:, 0:NTC], 15.5)
                for n in range(8):
                    y_n = y_all[:, n * NTC : (n + 1) * NTC]
                    nc.vector.scalar_tensor_tensor(
                        out=y_n, in0=xnib[:, n * NTC : (n + 1) * NTC],
                        scalar=-15.0, in1=chist[:, n * NTC : (n + 1) * NTC],
                        op0=AL.add, op1=AL.add,
                    )
                    nc.vector.tensor_scalar(
                        out=chist[:, (n + 1) * NTC : (n + 2) * NTC], in0=y_n,
                        scalar1=15.0, scalar2=16.0, op0=AL.max, op1=AL.min,
                    )

                # ---- phase 4: wrap to ymod in [0, 15.5] ----
                nc.vector.tensor_scalar(
                    out=wrap[:, :], in0=y_all[:, :], scalar1=15.75, scalar2=None,
                    op0=AL.is_ge,
                )
                nc.vector.scalar_tensor_tensor(
                    out=ymod[:, :], in0=wrap[:, :], scalar=-16.0, in1=y_all[:, :],
                    op0=AL.mult, op1=AL.add,
                )

                # ---- phase 5: soft dists s17 = relu(1 - |ymod - j|) ----
                # |.| and relu run on the otherwise-idle ACT engine
                G = 8 * NTC
                d_v = dtile[:, :].rearrange("p (g k) -> p g k", k=17)
                s_v = s17[:, :].rearrange("p (g k) -> p g k", k=17)
                ymod_b = ymod[:, :, None].broadcast_to([P, G, 17])
                iota_b = iota17[:, None, :].broadcast_to([P, G, 17])
                nc.vector.tensor_tensor(d_v, ymod_b, iota_b, op=AL.subtract)
                nc.scalar.activation(
                    dtile[:, :], dtile[:, :], mybir.ActivationFunctionType.Abs,
                )
                nc.scalar.activation(
                    s17[:, :], dtile[:, :], mybir.ActivationFunctionType.Relu,
                    bias=1.0, scale=-1.0,
                )
                # wraparound fold: dist[0] += s17[16]
                nc.vector.tensor_add(
                    s_v[:, :, 0:1], s_v[:, :, 0:1], s_v[:, :, 16:17]
                )

                # ---- phase 6: outer products + stores ----
                s5 = s17[:, :].rearrange(
                    "p (i two t k) -> p i two t k", two=2, t=NTC, k=17
                )
                for bt in range(nbat):
                    g = ch * nbat + bt
                    o4 = opool.tile([P, BATCH * F], FP, tag="o4")
                    for ti in range(BATCH):
                        t = bt * BATCH + ti
                        o_vv = o4[:, ti * F : (ti + 1) * F].rearrange(
                            "p (i h l) -> p i h l", h=16, l=16
                        )
                        h_b = s5[:, :, 1, t, 0:16][:, :, :, None].broadcast_to(
                            [P, 4, 16, 16])
                        l_b = s5[:, :, 0, t, 0:16][:, :, None, :].broadcast_to(
                            [P, 4, 16, 16])
                        eng = nc.gpsimd if ti < POOL_OUTER_PER_BATCH else nc.vector
                        eng.tensor_mul(o_vv, h_b, l_b)
                    o4_v = o4[:, :].rearrange("p (t f) -> p t f", t=BATCH)
                    nc.scalar.dma_start(out_v[g], o4_v)

    nc.finalize()
    return nc


_NC_CACHE = {}
LAST_RESULT = None


def kernel(**inputs) -> np.ndarray:
    global LAST_RESULT
    a = np.ascontiguousarray(np.asarray(inputs["a"], dtype=np.float32)).reshape(B_FULL, F)
    b = np.ascontiguousarray(np.asarray(inputs["b"], dtype=np.float32)).reshape(B_FULL, F)
    code2, iota17 = _const_tables()

    if ROWS not in _NC_CACHE:
        _NC_CACHE[ROWS] = build_nc(ROWS)
    nc = _NC_CACHE[ROWS]

    in_maps = []
    for c in range(N_CORES):
        in_maps.append({
            "a": a[c * ROWS : (c + 1) * ROWS],
            "b": b[c * ROWS : (c + 1) * ROWS],
            "code2": code2,
            "iota17": iota17,
        })
    res = run_bass_kernel_spmd(nc, in_maps, core_ids=list(range(N_CORES)))
    LAST_RESULT = res
    out = np.concatenate([r["out"] for r in res.results], axis=0)
    return out.reshape(B_FULL, 4, 256)
